# revision 8
# baseline (speedup 1.0000x reference)
"""IF spiking-neuron scan (charge / fire / hard-reset) on 8 Trainium2 cores.

Reference recurrence over t (elementwise on every [B, N] element):
    u_t = v_{t-1} + x_t          # charge
    s_t = (u_t >= 1.0)           # fire
    v_t = (1 - s_t) * u_t        # hard reset to 0

Sharding: pure data parallel over the B*N = 262144 element chains; each
of the 8 cores owns 32768 chains laid out as a [128, 256] tile per
timestep, with zero communication.

Kernel structure (one custom DVE op per timestep, spikes on ACT):

  The recurrence is rewritten on the u-sequence:
      u_{t+1} = (u_t if u_t < 1 else 0) + x_{t+1}
  which is ONE fused DVE instruction per timestep via a custom DVE op
  (registered at import):
      IF_STEP_ANT: out = select(Src0 < C0, Src0, Zero) + Src1
  This halves the vector-engine work vs the classic add + cmp/mult pair
  and keeps the pre-reset potential u_t materialized in SBUF, so the
  spike extraction runs on the otherwise-idle scalar engine off the
  critical path:
      r_t = Sign(1.0 - u_t)  ->  uint8
  r == 1 exactly when u < 1 (no spike); u >= 1 gives 0 or 255 (0.0 or
  -1.0 cast to uint8, saturate or wrap - both decode the same), so the
  host computes s = (r != 1). Sign(0) = 0 keeps u == V_TH exact.

  All fp32 arithmetic (the single add and the compare) is bit-identical
  to the reference. Input streams [P, t, F] on the sync-ring DMA queues,
  spike bytes return on the scalar-ring queues; with DVE at ~21 us and
  ACT at ~16 us the kernel is bound by the ~29 us of HBM traffic
  (8.4 MB in + 2.1 MB out per core at ~360 GB/s).
"""

import numpy as np

import concourse.tile as tile
from concourse import bacc, mybir
from concourse.bass_utils import run_bass_kernel_spmd

T = 64
B = 32
N = 8192
NCORES = 8
PERCORE = (B * N) // NCORES  # 32768 element chains per core
P = 128                      # SBUF partitions
F = PERCORE // P             # 256 elements per partition

V_TH = 1.0

# timestep blocks as (tb, f2): tb timesteps whose input arrives as one
# DMA with descriptors split f2 ways. The HW DGE assigns each
# 128-descriptor run (= one free-dim combo x 128 partitions) to ONE of
# the 16 ~22 GB/s DMA queues, so an unsplit (f2=1) timestep is a single
# 128 KiB / 5.9 us queue-latency unit. f2=2 halves that latency for the
# ramp blocks, where arrival latency gates the DVE start; steady-state
# blocks keep f2=1 (1 KiB descriptors at full bus efficiency, latency
# hidden by prefetch). The last block's spike extraction is split per
# TAIL_CHUNKS so the post-DVE tail is one short 1-timestep ACT + DMA.
BLOCKS = [(2, 2), (2, 2), (4, 2)] + [(8, 1)] * 7
assert sum(tb for tb, _ in BLOCKS) == T
TAIL_CHUNKS = [5, 1, 1, 1]  # ACT/output sub-chunks of the final block
assert sum(TAIL_CHUNKS) == BLOCKS[-1][0]

_NC_CACHE = {}
_OP_CACHE = {}


def _register_if_step_op():
    """Register the fused IF-neuron step as a custom DVE op.

    Uses the documented extension point (concourse.dve_ops.OPS): the op
    body lowers to a single steady-state uop program whose sha is pinned
    at registration, the sub-opcode row is taken from the free range
    [1, 0x20), and the numpy reference makes CoreSim scheduling exact.
    """
    if "op" in _OP_CACHE:
        return _OP_CACHE["op"]

    import concourse.dve_ops as dve_ops
    from concourse.dve_spec import Spec, Src0, Src1, C0, Zero, select, lower, _has_src1
    from concourse.dve_uop import DveOpSpec

    name = "IF_STEP_ANT"

    def _ref(in0, in1, c0, c1, c2):
        u = np.where(
            in0.astype(np.float32) < np.float32(c0),
            in0.astype(np.float32),
            np.float32(0.0),
        ).astype(np.float32)
        return (u + in1.astype(np.float32)).astype(np.float32)

    spec = Spec(body=select(Src0 < C0, Src0, Zero) + Src1, reference=_ref)

    existing = {op.name: op for op in dve_ops.OPS}
    if name in existing:
        _OP_CACHE["op"] = existing[name]
        return existing[name]

    row = 1 + len(dve_ops.OPS)
    shas = {}
    for ver in ("v3", "v4"):
        try:
            uops = lower(spec, ver=ver)
            shas[ver] = DveOpSpec(
                name=name, opcode=row, uops=uops, rd1_en=_has_src1(spec)
            ).sha(ver)
        except Exception:
            pass  # ver not supported in this build; TRN2 only needs v3

    op = dve_ops.DveOp(name, spec, subdim=False, uops_sha=shas)
    dve_ops.OPS.append(op)
    dve_ops._SUB_OPCODE_FOR_NAME[name] = row
    dve_ops.CUSTOM_DVE_SPECS[name] = spec
    _OP_CACHE["op"] = op
    return op


def build_nc(blocks=None):
    blocks = list(BLOCKS if blocks is None else blocks)
    if_step = _register_if_step_op()
    # Bacc (not raw Bass): its compile() splits multi-wait sync conditions
    # into nop/event-semaphore prefixes — walrus accepts at most one sync
    # wait per hardware instruction.
    nc = bacc.Bacc("TRN2", target_bir_lowering=False, debug=False)
    x = nc.dram_tensor("x", [T, PERCORE], mybir.dt.float32, kind="ExternalInput").ap()
    y = nc.dram_tensor("y", [P, T, F], mybir.dt.uint8, kind="ExternalOutput").ap()

    # x: [T, P*F] -> [P, T, F]; per partition each timestep is a
    # contiguous 1 KiB run in DRAM (f2=1 blocks DMA straight from this).
    xr = x.rearrange("t (p f) -> p t f", p=P)

    with tile.TileContext(nc) as tc:
        with (
            tc.tile_pool(name="xin", bufs=4) as xpool,
            tc.tile_pool(name="ub", bufs=4) as upool,
            tc.tile_pool(name="sout", bufs=4) as spool,
            tc.tile_pool(name="z", bufs=1) as zpool,
        ):
            zero = zpool.tile([P, F], mybir.dt.float32)
            nc.vector.memset(zero[:], 0.0)
            prev = zero  # tile holding u_{t-1} in its last F-slice
            prev_lo = 0
            t0 = 0
            for bi, (tb, f2) in enumerate(blocks):
                xt = xpool.tile([P, tb * F], mybir.dt.float32, tag="xin")
                if f2 == 1:
                    nc.sync.dma_start(xt[:], xr[:, t0:t0 + tb, :])
                else:
                    # f2-split ramp block: SBUF holds [f2][t][f1] per
                    # partition so descriptors are F/f2 elements and each
                    # timestep's input spreads over f2 DMA queues.
                    f1 = F // f2
                    xv = xt[:].rearrange(
                        "p (f2 t f1) -> p f2 t f1", f2=f2, t=tb, f1=f1
                    )
                    xd = x.rearrange(
                        "t (p f2 f1) -> p f2 t f1", p=P, f2=f2, f1=f1
                    )
                    nc.sync.dma_start(xv, xd[:, :, t0:t0 + tb, :])
                ub = upool.tile([P, tb * F], mybir.dt.float32, tag="ub")
                for ti in range(tb):
                    lo = ti * F
                    if f2 == 1:
                        x_in = xt[:, lo:lo + F]
                        u_out = ub[:, lo:lo + F]
                        u_in = prev[:, prev_lo:prev_lo + F]
                    else:
                        # timestep ti's input is strided across the f2
                        # groups; give all operands the same [P, f2, f1]
                        # free shape (u slices are contiguous views).
                        f1 = F // f2
                        x_in = xt[:].rearrange(
                            "p (f2 t f1) -> p f2 t f1", f2=f2, t=tb, f1=f1
                        )[:, :, ti, :]
                        u_out = ub[:, lo:lo + F].rearrange(
                            "p (f2 f1) -> p f2 f1", f2=f2, f1=f1
                        )
                        u_in = prev[:, prev_lo:prev_lo + F].rearrange(
                            "p (f2 f1) -> p f2 f1", f2=f2, f1=f1
                        )
                    nc.vector._custom_dve(
                        if_step,
                        out=u_out,
                        in0=u_in,
                        in1=x_in,
                        s0=V_TH,
                    )
                    prev, prev_lo = ub, lo
                # r = Sign(V_TH - u) cast to uint8: 1 <=> no spike; spike
                # rows are 0 (u == V_TH) or the cast of -1.0 (saturate 0 /
                # wrap 255). Host decodes s = (r != 1); Sign(0) = 0 keeps
                # exact threshold ties correct. The final block emits its
                # spikes in sub-chunks so the post-DVE tail is one short
                # 1-timestep ACT + DMA instead of a whole block's.
                chunks = TAIL_CHUNKS if bi == len(blocks) - 1 else [tb]
                c0 = 0
                for ct in chunks:
                    st = spool.tile([P, ct * F], mybir.dt.uint8, tag="sout")
                    nc.scalar.activation(
                        st[:], ub[:, c0 * F:(c0 + ct) * F],
                        mybir.ActivationFunctionType.Sign,
                        bias=V_TH, scale=-1.0,
                    )
                    # outputs ride the scalar engine's HW-DGE ring so input
                    # triggers never queue behind them on the SP ring
                    nc.scalar.dma_start(y[:, t0 + c0:t0 + c0 + ct, :], st[:])
                    c0 += ct
                t0 += tb
    nc.compile()
    return nc


def _get_nc():
    if "nc" not in _NC_CACHE:
        _NC_CACHE["nc"] = build_nc()
    return _NC_CACHE["nc"]


def run_sharded(x_seq, trace=False, nc=None, **kwargs):
    if nc is None:
        nc = _get_nc()
    x2 = np.ascontiguousarray(np.asarray(x_seq, dtype=np.float32)).reshape(T, B * N)
    in_maps = [
        {"x": np.ascontiguousarray(x2[:, c * PERCORE:(c + 1) * PERCORE])}
        for c in range(NCORES)
    ]
    # A cold device occasionally reports NRT_EXEC_UNIT_UNRECOVERABLE on the
    # first execute and recovers on the next attempt; retry a couple times.
    for attempt in range(3):
        try:
            res = run_bass_kernel_spmd(
                nc, in_maps, list(range(NCORES)), trace=trace, **kwargs
            )
            break
        except Exception:  # jax.errors.JaxRuntimeError and friends
            if attempt == 2:
                raise
            import time
            time.sleep(2.0)
    out = np.empty((T, B * N), dtype=np.float32)
    for c in range(NCORES):
        yc = np.asarray(res.results[c]["y"])          # [P, T, F] uint8
        r = yc.transpose(1, 0, 2).reshape(T, PERCORE)
        # r == 1 <=> no spike (u < V_TH); 0 and 255 both mean spike
        out[:, c * PERCORE:(c + 1) * PERCORE] = (r != 1)
    return out.reshape(T, B, N), res


def kernel(x_seq):
    out, _ = run_sharded(x_seq)
    return out
